# revision 5
# baseline (speedup 1.0000x reference)
"""ConvexSampler Trainium2 kernel.

convex[k] = s[k] * z[idx_i[k]] + (1 - s[k]) * z[idx_j[k]]
out = (concat([z, convex], 0), concat([label_ids, full(UNSEEN)], 0))

Strategy (8 NeuronCores, data-parallel over the 32768 convex samples):
  - core c computes convex rows [c*4096, (c+1)*4096)
  - z (8192x768 f32) is replicated to every core's HBM
  - per core: rows are gathered from z with indirect (SWDGE) DMA,
    K rows per SBUF partition per gather instruction, axpy on DVE,
    stored back with HWDGE DMA.
Row->partition layout is "transposed": partition p of tile t holds convex
rows p*COLS + t*K + c (c in [0,K)), which makes every index/scalar load a
single fully-contiguous DMA and keeps stores at 3KB+ per descriptor.
"""

import os
import numpy as np

import concourse.bacc as bacc
import concourse.bass as bass
import concourse.mybir as mybir
import concourse.tile as tile
from concourse.bass_utils import run_bass_kernel_spmd

N_CORES = 8
BATCH = 8192
FEAT = 768
NUM_CONVEX = 32768
SHARD = NUM_CONVEX // N_CORES  # 4096
P = 128
COLS = SHARD // P  # 32 convex rows per partition per core
UNSEEN_LABEL_ID = 150

# rows gathered per partition per indirect-DMA instruction.
# NOTE: hardware indirect DMA only honors ONE index per partition per
# instruction (offset AP must be [P, 1]) — K must stay 1.
K = int(os.environ.get("CONVEX_K", "1"))
WORK_BUFS = int(os.environ.get("CONVEX_BUFS", "6"))

_f32 = mybir.dt.float32
_i32 = mybir.dt.int32


def build_nc(k=K, work_bufs=WORK_BUFS):
    nc = bacc.Bacc("TRN2", target_bir_lowering=False, debug=False)

    z = nc.dram_tensor("z", [BATCH, FEAT], _f32, kind="ExternalInput").ap()
    ii = nc.dram_tensor("idx_i", [SHARD], _i32, kind="ExternalInput").ap()
    jj = nc.dram_tensor("idx_j", [SHARD], _i32, kind="ExternalInput").ap()
    ss = nc.dram_tensor("s", [SHARD], _f32, kind="ExternalInput").ap()
    out = nc.dram_tensor("convex", [SHARD, FEAT], _f32, kind="ExternalOutput").ap()

    n_tiles = COLS // k
    out3d = out.rearrange("(p n) d -> p n d", p=P)

    with tile.TileContext(nc) as tc:
        with (
            tc.tile_pool(name="idx", bufs=1) as idxp,
            tc.tile_pool(name="work", bufs=work_bufs) as wp,
        ):
            ii_sb = idxp.tile([P, COLS], _i32)
            jj_sb = idxp.tile([P, COLS], _i32)
            ss_sb = idxp.tile([P, COLS], _f32)
            nc.sync.dma_start(out=ii_sb[:, :], in_=ii.rearrange("(p n) -> p n", p=P))
            nc.sync.dma_start(out=jj_sb[:, :], in_=jj.rearrange("(p n) -> p n", p=P))
            nc.sync.dma_start(out=ss_sb[:, :], in_=ss.rearrange("(p n) -> p n", p=P))
            if k == 1:
                oms_sb = idxp.tile([P, COLS], _f32)  # 1 - s
                nc.vector.tensor_scalar(
                    out=oms_sb[:, :], in0=ss_sb[:, :], scalar1=-1.0, scalar2=1.0,
                    op0=mybir.AluOpType.mult, op1=mybir.AluOpType.add,
                )

            for t in range(n_tiles):
                zi = wp.tile([P, k * FEAT], _f32, tag="zi")
                zj = wp.tile([P, k * FEAT], _f32, tag="zj")
                ot = wp.tile([P, k * FEAT], _f32, tag="ot")
                csl = slice(t * k, (t + 1) * k)
                zi_dst = zi[:, :] if k == 1 else zi[:, :].rearrange("p (k d) -> p k d", k=k)
                zj_dst = zj[:, :] if k == 1 else zj[:, :].rearrange("p (k d) -> p k d", k=k)
                nc.gpsimd.indirect_dma_start(
                    out=zi_dst, out_offset=None, in_=z,
                    in_offset=bass.IndirectOffsetOnAxis(ap=ii_sb[:, csl], axis=0),
                )
                nc.gpsimd.indirect_dma_start(
                    out=zj_dst, out_offset=None, in_=z,
                    in_offset=bass.IndirectOffsetOnAxis(ap=jj_sb[:, csl], axis=0),
                )
                if k == 1:
                    # exact reference expression: s*zi + (1-s)*zj
                    nc.vector.tensor_scalar(
                        out=ot[:, :], in0=zj[:, :],
                        scalar1=oms_sb[:, csl], scalar2=None,
                        op0=mybir.AluOpType.mult,
                    )
                    nc.vector.scalar_tensor_tensor(
                        out=ot[:, :], in0=zi[:, :], scalar=ss_sb[:, csl],
                        in1=ot[:, :], op0=mybir.AluOpType.mult,
                        op1=mybir.AluOpType.add,
                    )
                else:
                    # zj + s*(zi - zj), with s broadcast over the feature dim
                    s_b = ss_sb[:, csl].unsqueeze(-1).to_broadcast([P, k, FEAT])
                    zi3 = zi[:, :].rearrange("p (k d) -> p k d", k=k)
                    nc.vector.tensor_sub(zi[:, :], zi[:, :], zj[:, :])
                    nc.vector.tensor_tensor(
                        out=zi3, in0=zi3, in1=s_b, op=mybir.AluOpType.mult
                    )
                    nc.vector.tensor_add(ot[:, :], zi[:, :], zj[:, :])
                nc.sync.dma_start(
                    out=out3d[:, csl, :],
                    in_=ot[:, :].rearrange("p (k d) -> p k d", k=k),
                )
    nc.finalize()
    return nc


_NC_CACHE = {}


def _get_nc():
    key = (K, WORK_BUFS)
    if key not in _NC_CACHE:
        _NC_CACHE[key] = build_nc(*key)
    return _NC_CACHE[key]


def kernel(z, label_ids, idx_i, idx_j, s, **bass_run_kwargs):
    z = np.ascontiguousarray(np.asarray(z, dtype=np.float32))
    label_ids = np.asarray(label_ids)
    ii = np.ascontiguousarray(np.asarray(idx_i, dtype=np.int32))
    jj = np.ascontiguousarray(np.asarray(idx_j, dtype=np.int32))
    sv = np.ascontiguousarray(np.asarray(s, dtype=np.float32))

    nc = _get_nc()
    in_maps = []
    for c in range(N_CORES):
        sl = slice(c * SHARD, (c + 1) * SHARD)
        in_maps.append({"z": z, "idx_i": ii[sl], "idx_j": jj[sl], "s": sv[sl]})

    res = run_bass_kernel_spmd(
        nc, in_maps, core_ids=list(range(N_CORES)), **bass_run_kwargs
    )
    convex = np.concatenate([r["convex"] for r in res.results], axis=0)

    z_out = np.concatenate([z, convex], axis=0)
    labels_out = np.concatenate(
        [label_ids, np.full((NUM_CONVEX,), UNSEEN_LABEL_ID, dtype=label_ids.dtype)]
    )
    if "exec_time_ns" in dir(res):
        kernel.last_results = res
    return z_out, labels_out


kernel.last_results = None
